# revision 25
# baseline (speedup 1.0000x reference)
"""FM bi-interaction (embedding_lookup) Trainium2 kernel — v2.

out[n, k] = 0.5 * ((x @ E)^2 - (x*x) @ (E*E))[n, k] * mask[n]
mask[n] = 1 if n in train_idx else 0

Strategy (all sharding/prep is host-side, inside kernel()):
- Only rows that appear in train_idx produce nonzero output (~55% of rows
  for the target distribution: 16000 draws with replacement from 20000).
  Dedup train_idx, gather just those rows, and scatter results back into a
  zero output. The on-device mask disappears entirely.
- The 0.5 factor and the mask are folded into the embedding table: with
  E' = sqrt(0.5) * E, (x@E')^2 - (x*x)@(E'*E') = 0.5*((x@E)^2 - (x*x)@(E*E)).
- x is uploaded in bf16 (halves HBM traffic; rel-err ~3e-3 << 2e-2 budget)
  in f-major [F, R_pad] layout per core so every x DMA is a ~1.4 MB
  transfer with ~2.8 KB contiguous lines.
- PE: the K=32 output only fills 32 of 128 PE columns, so four matmuls run
  concurrently via col-tiling (tile_position=(0, 32j)): groups 0/1 hold
  L/R partial sums for even f-tile pairs, groups 2/3 for odd pairs. The
  epilogue adds the two partial L's (and R's), then out = L^2 - R.
- x^2 is computed on device, split between VectorE and ScalarE.

Rows per core are padded to a multiple of 128; output PSUM banks cover 512
columns each. The Bass program is cached per padded row count.
"""

import math
import sys

if "/opt/trn_rl_repo" not in sys.path:
    sys.path.insert(0, "/opt/trn_rl_repo")

import numpy as np
import ml_dtypes

BF16 = ml_dtypes.bfloat16

N_ROWS = 20000
F = 10000
EK = 32
CORES = 8
FP = 125  # contraction rows per f-tile
FTILES = F // FP  # 80
QUAD = 4
NT = 8  # f-tiles per DMA transfer (2 quads)
NXF = FTILES // NT  # 10 transfers per iteration
NCHUNK = 512  # output columns per PSUM bank
CPAD = 128  # per-core row count is padded to a multiple of this

_PROGRAM_CACHE: dict = {}


def _build_program(rpad: int, repeats: int = 1):
    """Per-core Bass program for rpad gathered rows (multiple of CPAD).

    repeats > 1 unrolls the whole pipeline R times inside one NEFF (same
    inputs, same outputs, recomputed each repeat) — used only by test.py to
    measure steady-state per-iteration device time as
    (t(R_hi) - t(R_lo)) / (R_hi - R_lo), which cancels all dispatch and
    NEFF-launch overheads.
    """
    import concourse.mybir as mybir
    import concourse.tile as tile
    from concourse import bacc

    f32 = mybir.dt.float32
    bf16 = mybir.dt.bfloat16
    # output chunks: full 512-col PSUM banks plus one partial bank
    chunk_cols = [NCHUNK] * (rpad // NCHUNK)
    if rpad % NCHUNK:
        chunk_cols.append(rpad % NCHUNK)
    nch = len(chunk_cols)
    ps_bufs = 2 if 2 * nch <= 8 else 1

    nc = bacc.Bacc("TRN2", target_bir_lowering=False, debug=False)
    # f-tile-blocked layout: xt[p, t*rpad + n] = x[t*FP + p, n]. Each quad
    # DMA reads ONE contiguous QUAD*rpad*2-byte line per partition (~11 KB)
    # instead of four strided ~2.8 KB lines — measured ~1.7x DMA bandwidth.
    xt = nc.dram_tensor("xt", [FP, FTILES * rpad], bf16, kind="ExternalInput")
    embP = nc.dram_tensor("embP", [FP, FTILES * EK], bf16, kind="ExternalInput")
    outT = nc.dram_tensor("outT", [EK, rpad], f32, kind="ExternalOutput")

    with tile.TileContext(nc) as tc:
        with (
            tc.tile_pool(name="wpool", bufs=1) as wpool,
            tc.tile_pool(name="xpool", bufs=3) as xpool,
            tc.tile_pool(name="qpool", bufs=3) as qpool,
            tc.tile_pool(name="opool", bufs=2) as opool,
            tc.tile_pool(name="pspool", bufs=1, space="PSUM") as pspool,
        ):
            # Embedding table, pre-scaled by sqrt(0.5) and pre-rearranged to
            # [FP, FTILES*EK] on host: one fully contiguous DMA. Issued on
            # the (otherwise idle) SWDGE queue so it doesn't delay the xt
            # prefetch streams on the two HWDGE queues.
            e_sb = wpool.tile([FP, FTILES * EK], bf16)
            nc.gpsimd.dma_start(out=e_sb[:], in_=embP[:])
            e2_sb = wpool.tile([FP, FTILES * EK], bf16)
            nc.vector.tensor_mul(e2_sb[:], e_sb[:], e_sb[:])

            def wslice(sb, t):
                return sb[:, t * EK : (t + 1) * EK]

            for rep in range(repeats):
                # One PSUM bank per chunk; partition groups hold the four
                # col-tiled accumulators: [0:32]=L(t%4 in 0,1), [32:64]=R
                # (same), [64:96]=L(t%4 in 2,3), [96:128]=R(same).
                ps = [
                    pspool.tile(
                        [128, NCHUNK], f32, space="PSUM", name=f"ps{c}", bufs=ps_bufs
                    )
                    for c in range(nch)
                ]

                for xf in range(NXF):
                    t0 = NT * xf
                    # 2D tile + flat DRAM slice: one contiguous ~22 KB
                    # descriptor per partition (a 3D "p (a n) -> p a n" AP
                    # splits each partition into NT separate descriptors
                    # and caps DMA throughput at ~200 GB/s).
                    xt_sb = xpool.tile([FP, NT * rpad], bf16, name="xt_sb")
                    # Alternate the two HWDGE queues: one queue's per-DMA
                    # fixed cost hides under the other queue's data movement.
                    dma_eng = nc.sync if xf % 2 == 0 else nc.scalar
                    dma_eng.dma_start(
                        out=xt_sb[:], in_=xt[:, t0 * rpad : (t0 + NT) * rpad]
                    )
                    xq_sb = qpool.tile([FP, NT * rpad], bf16, name="xq_sb")
                    # split the squaring 3:1 across VectorE and ScalarE
                    # (DVE gets 2x bf16 throughput; ScalarE does not)
                    nc.vector.tensor_mul(
                        xq_sb[:, 0 : 6 * rpad],
                        xt_sb[:, 0 : 6 * rpad],
                        xt_sb[:, 0 : 6 * rpad],
                    )
                    nc.scalar.square(
                        xq_sb[:, 6 * rpad : 8 * rpad], xt_sb[:, 6 * rpad : 8 * rpad]
                    )

                    def mslice(sb, a, c):
                        lo = a * rpad + c * NCHUNK
                        return sb[:, lo : lo + chunk_cols[c]]

                    for qq in range(NT // QUAD):
                        q0 = QUAD * qq
                        for c in range(nch):
                            cs = slice(0, chunk_cols[c])
                            for h in range(2):
                                t = t0 + q0 + h  # even pair -> groups 0, 1
                                u = t0 + q0 + 2 + h  # odd pair -> groups 2, 3
                                nc.tensor.matmul(
                                    ps[c][0:32, cs],
                                    wslice(e_sb, t),
                                    mslice(xt_sb, q0 + h, c),
                                    start=(t == 0),
                                    stop=(t == FTILES - 3),
                                    tile_position=(0, 0),
                                    skip_group_check=True,
                                )
                                nc.tensor.matmul(
                                    ps[c][32:64, cs],
                                    wslice(e2_sb, t),
                                    mslice(xq_sb, q0 + h, c),
                                    start=(t == 0),
                                    stop=(t == FTILES - 3),
                                    tile_position=(0, 32),
                                    skip_group_check=True,
                                )
                                nc.tensor.matmul(
                                    ps[c][64:96, cs],
                                    wslice(e_sb, u),
                                    mslice(xt_sb, q0 + 2 + h, c),
                                    start=(u == 2),
                                    stop=(u == FTILES - 1),
                                    tile_position=(0, 64),
                                    skip_group_check=True,
                                )
                                nc.tensor.matmul(
                                    ps[c][96:128, cs],
                                    wslice(e2_sb, u),
                                    mslice(xq_sb, q0 + 2 + h, c),
                                    start=(u == 2),
                                    stop=(u == FTILES - 1),
                                    tile_position=(0, 96),
                                    skip_group_check=True,
                                )

                # Epilogue: L = g0 + g2, R = g1 + g3, out = L*L - R.
                for c in range(nch):
                    ns = slice(c * NCHUNK, c * NCHUNK + chunk_cols[c])
                    cs = slice(0, chunk_cols[c])
                    lsb = opool.tile([EK, NCHUNK], f32, name="lsb")
                    nc.scalar.activation(
                        lsb[:, cs], ps[c][0:32, cs], mybir.ActivationFunctionType.Copy
                    )
                    nc.vector.tensor_add(lsb[:, cs], lsb[:, cs], ps[c][64:96, cs])
                    rsb = opool.tile([EK, NCHUNK], f32, name="rsb")
                    nc.scalar.activation(
                        rsb[:, cs], ps[c][32:64, cs], mybir.ActivationFunctionType.Copy
                    )
                    nc.vector.tensor_add(rsb[:, cs], rsb[:, cs], ps[c][96:128, cs])
                    osb = opool.tile([EK, NCHUNK], f32, name="osb")
                    nc.scalar.square(osb[:, cs], lsb[:, cs])
                    nc.vector.tensor_sub(osb[:, cs], osb[:, cs], rsb[:, cs])
                    # outT goes out via SWDGE (Pool engine, otherwise idle)
                    # so it never blocks either xt prefetch queue at
                    # iteration (and dispatch) boundaries.
                    nc.gpsimd.dma_start(out=outT[:, ns], in_=osb[:, cs])

    nc.compile()
    return nc


def _get_program(rpad: int, repeats: int = 1):
    key = (rpad, repeats)
    if key not in _PROGRAM_CACHE:
        _PROGRAM_CACHE[key] = _build_program(rpad, repeats)
    return _PROGRAM_CACHE[key]


def _prepare_in_maps(input, emb_weight, train_idx):
    x = np.asarray(input, dtype=np.float32)
    e = np.asarray(emb_weight, dtype=np.float32)
    idx = np.asarray(train_idx).astype(np.int64)

    uniq = np.unique(idx)
    u = len(uniq)
    per_core = max(1, math.ceil(u / CORES))
    rpad = CPAD * math.ceil(per_core / CPAD)

    # embedding: scale by sqrt(0.5) (folds the 0.5 and keeps L^2-R exact),
    # rearrange to [FP, FTILES*EK] so the device DMA is contiguous.
    es = (e * math.sqrt(0.5)).reshape(FTILES, FP, EK).transpose(1, 0, 2)
    embP = np.ascontiguousarray(es.reshape(FP, FTILES * EK).astype(BF16))

    groups = []
    in_maps = []
    for c in range(CORES):
        sel = uniq[c * per_core : (c + 1) * per_core]
        groups.append(sel)
        # blocked layout: xt[p, t, n] = x_gathered[n, t*FP + p]
        xt = np.zeros((FP, FTILES, rpad), dtype=BF16)
        if len(sel):
            xg = x[sel].astype(BF16)  # [cnt, F]
            xt[:, :, : len(sel)] = xg.T.reshape(FTILES, FP, len(sel)).transpose(
                1, 0, 2
            )
        in_maps.append({"xt": xt.reshape(FP, FTILES * rpad), "embP": embP})
    return in_maps, rpad, groups


def run_sharded(input, emb_weight, train_idx, trace: bool = False):
    """Run on 8 cores; returns (full_output, BassKernelResults)."""
    from concourse.bass_utils import run_bass_kernel_spmd

    in_maps, rpad, groups = _prepare_in_maps(input, emb_weight, train_idx)
    nc = _get_program(rpad)
    res = run_bass_kernel_spmd(nc, in_maps, core_ids=list(range(CORES)), trace=trace)
    out = np.zeros((N_ROWS, EK), dtype=np.float32)
    for c in range(CORES):
        sel = groups[c]
        if len(sel):
            out[sel, :] = res.results[c]["outT"].T[: len(sel)]
    return out, res


def kernel(input, emb_weight, train_idx):
    out, _ = run_sharded(input, emb_weight, train_idx)
    return out
